# revision 33
# baseline (speedup 1.0000x reference)
"""3-layer GCN (ContrastiveGNN) on 8 Trainium2 NeuronCores.

Strategy (dst-sharded edge partition, "1D graph partition"):
  - Nodes keep their original device (dev = node // 12500); within each device
    nodes are snake-dealt by in-degree into 17 batch bins (16 x 768 + 1 x 256
    slots) so per-(batch, src-pair) edge counts are balanced across devices.
  - Math reorder: for each GCN layer,
        out = D^-1/2 (A+I) D^-1/2 (h W) + b  ==  dis_d * (sum_{e->d} T[src]) @ W + b
    with T = dis * h (row-scaled activations).  Aggregation happens BEFORE the
    dense transform, so the gather tables carry 128 features for every layer.
  - Self-loops are NOT gathered: the self term dis_d*h[d] == T[d] is added into
    PSUM with one identity matmul per window, reading the device's own T rows
    (layer 1: a dedicated per-device input; layers 2-3: the resident stage tile
    holding the previous layer's dis-scaled output).
  - Aggregation on the tensor engine: per (batch, pair) all edges are packed
    densely sorted by dst window (NO per-window padding; groups of 128 edges
    may span window boundaries).  Each (group, window) intersection contributes
    one_hot[e, dst_in_batch==iota_w].T @ gathered[e, feat] accumulated in PSUM
    per 128-dst window.  One-hots are built on DVE via fp16 is_equal against
    per-window iota rows (values w_rel*128..w_rel*128+127).
  - Gathers use the SWDGE dma_gather custom instruction (int16 indices =>
    the 100352-row table is addressed in 4 block-pair regions of 25088 rows).
    One dma_gather per (batch, pair); pair p runs on SWDGE queue p so the four
    descriptor rings drain in parallel.
  - Tables are bf16 (PSUM accumulation f32); between layers the 8 per-device
    table blocks are exchanged with an AllGather collective.
  - All 8 devices run one SPMD program: per-call slot counts are padded to the
    max over devices, so instruction streams are identical and only the input
    data (indices, dst positions, dis) differs.
"""

import numpy as np
import ml_dtypes

BF16 = ml_dtypes.bfloat16
FP16 = np.float16

N = 100000
F = 128
DOUT = 64
M = 8
BLK = N // M            # 12500 dst nodes per device
P = 128
WPD = 98                # windows per device (98*128 = 12544)
BLKP = WPD * P          # 12544 padded block rows
TROWS = M * BLKP        # 100352 table rows
PAIR = 2 * BLKP         # 25088 rows per src-block-pair region (int16-addressable)
NPAIR = 4
WB = 6                  # windows per batch bin
NBATCH = 17             # 16 full bins (768 slots) + one 256-slot bin
BINFULL = WB * P        # 768
PADREL = PAIR - 1       # pair-local row of a guaranteed-zero table row


class _Call:
    __slots__ = ("ic0", "dc0", "c16", "c128", "nslots", "slots", "ohw")


def _preprocess(x, edge_index, W1, b1, W2, b2, W3, b3):
    """Host-side index plumbing + input staging."""
    x = np.asarray(x, np.float32)
    ei = np.asarray(edge_index)
    src = ei[0].astype(np.int64)
    dst = ei[1].astype(np.int64)

    deg = (np.bincount(dst, minlength=N) + 1).astype(np.float32)  # +1 self-loop
    dis = (1.0 / np.sqrt(deg)).astype(np.float32)

    # --- balanced slot assignment: dev fixed, snake-deal by degree into bins
    slot_in_block = np.empty(N, np.int64)
    nfull = 16 * BINFULL  # 12288
    for i in range(M):
        nodes = np.arange(i * BLK, (i + 1) * BLK)
        order = nodes[np.argsort(-deg[nodes], kind="stable")]
        main, tail = order[:nfull], order[nfull:]
        k = np.arange(nfull)
        fwd = (k // 16) % 2 == 0
        b = np.where(fwd, k % 16, 15 - (k % 16))
        slot_in_block[main] = b * BINFULL + (k // 16)
        slot_in_block[tail] = nfull + np.arange(len(tail))
    # lane-major table rows: row = lane*WPD + w, so the on-device stage tile
    # [P, WPD*F] maps to the table block with a straight contiguous copy.
    trow = (np.arange(N) // BLK) * BLKP + (slot_in_block % P) * WPD + (
        slot_in_block // P
    )

    # --- gather table for layer 1: dis-scaled input features
    xs = (x * dis[:, None]).astype(BF16)
    T1 = np.zeros((TROWS, F), BF16)
    T1[trow] = xs

    # --- edge bucketing (no self loops)
    e_dev = dst // BLK
    d_slot = slot_in_block[dst]
    e_batch = d_slot // BINFULL          # 0..16
    dl_all = d_slot - e_batch * BINFULL  # 0..767
    e_pair = (src // BLK) // 2
    rel_all = (trow[src] - e_pair * PAIR).astype(np.int64)
    bkey = e_batch * NPAIR + e_pair      # 0..67
    NBUCK = NBATCH * NPAIR

    cnt = np.zeros((M, NBUCK), np.int64)
    for i in range(M):
        cnt[i] = np.bincount(bkey[e_dev == i], minlength=NBUCK)
    cap = np.maximum(128, -(-cnt.max(axis=0) // P) * P)  # per-bucket slots

    # per-device window start/end within each bucket (slot positions)
    # windows per batch bin: WB except last bin (2)
    wcnt_of = [WB] * 16 + [2]

    meta_calls = {}
    ic = dc = off = 0
    win_starts = np.zeros((M, NBUCK, WB), np.int64)
    win_ends = np.zeros((M, NBUCK, WB), np.int64)

    # sort edges per device by (bucket, dl)
    dev_data = []
    for i in range(M):
        m = e_dev == i
        key = bkey[m] * BINFULL + dl_all[m]
        o = np.argsort(key, kind="stable")
        bk_s = bkey[m][o]
        dl_s = dl_all[m][o]
        rel_s = rel_all[m][o]
        starts = np.searchsorted(bk_s, np.arange(NBUCK))
        ends = np.searchsorted(bk_s, np.arange(NBUCK), side="right")
        for bu in range(NBUCK):
            s0, s1 = starts[bu], ends[bu]
            dseg = dl_s[s0:s1]
            wc = wcnt_of[bu // NPAIR]
            for w in range(wc):
                win_starts[i, bu, w] = np.searchsorted(dseg, w * P)
                win_ends[i, bu, w] = np.searchsorted(dseg, (w + 1) * P)
        dev_data.append((bk_s, dl_s, rel_s, starts, ends))

    # union group ranges + call metadata (SPMD-uniform)
    nmm = np.ones(WPD, np.int64)  # 1 self-matmul per window
    for wb in range(NBATCH):
        wc = wcnt_of[wb]
        for p_ in range(NPAIR):
            bu = wb * NPAIR + p_
            c = _Call()
            c.ic0, c.dc0 = ic, dc
            c.nslots = int(cap[bu])
            c.c16 = c.nslots // 16
            c.c128 = c.nslots // P
            slots = []
            base = 0
            for w in range(wc):
                glo = int(min(win_starts[i2, bu, w] for i2 in range(M)) // P)
                ghi_e = max(int(win_ends[i2, bu, w]) for i2 in range(M))
                ghi = min(-(-ghi_e // P), c.c128)
                ghi = max(ghi, glo)
                if ghi > glo:
                    slots.append((w, glo, ghi, base))
                    base += ghi - glo
                    nmm[wb * WB + w] += ghi - glo
            c.slots = slots
            c.ohw = base
            ic += c.c16
            dc += c.c128
            off += c.nslots
            meta_calls[(wb, p_)] = c
    tot_slots = off

    meta = {
        "calls": meta_calls,
        "nmm": nmm,
        "sc16": tot_slots // 16,
        "sc128": tot_slots // P,
        "tot_slots": tot_slots,
    }

    # --- per-device padded slot arrays
    iota_np = np.zeros((P, WB, P), FP16)
    for j in range(WB):
        iota_np[:, j, :] = np.arange(j * P, (j + 1) * P, dtype=np.float32)[None, :]
    ident_np = np.eye(P, dtype=np.float32).astype(BF16)
    onesr_np = np.zeros((P, P), BF16)
    onesr_np[0, :] = 1
    w1b = np.asarray(W1, np.float32).astype(BF16)
    w2b = np.asarray(W2, np.float32).astype(BF16)
    w3b = np.asarray(W3, np.float32).astype(BF16)
    # bias as a rank-1 matmul operand: row 0 holds b, other rows zero
    b1f = np.zeros((P, F), np.float32).astype(BF16)
    b1f[0] = np.asarray(b1, np.float32)
    b2f = np.zeros((P, F), np.float32).astype(BF16)
    b2f[0] = np.asarray(b2, np.float32)
    b3f = np.zeros((P, DOUT), np.float32).astype(BF16)
    b3f[0] = np.asarray(b3, np.float32)

    in_maps = []
    for i in range(M):
        bk_s, dl_s, rel_s, starts, ends = dev_data[i]
        idxfl = np.full(tot_slots, PADREL, np.int16)
        dlfl = np.full(tot_slots, -1.0, np.float32)
        off2 = 0
        for wb in range(NBATCH):
            for p_ in range(NPAIR):
                bu = wb * NPAIR + p_
                c = meta_calls[(wb, p_)]
                s0, s1 = starts[bu], ends[bu]
                n = s1 - s0
                idxfl[off2 : off2 + n] = rel_s[s0:s1].astype(np.int16)
                dlfl[off2 : off2 + n] = dl_s[s0:s1]
                off2 += c.nslots

        i16_parts, d128_parts = [], []
        off2 = 0
        for wb in range(NBATCH):
            for p_ in range(NPAIR):
                c = meta_calls[(wb, p_)]
                seg_i = idxfl[off2 : off2 + c.nslots]
                seg_d = dlfl[off2 : off2 + c.nslots]
                i16_parts.append(seg_i.reshape(-1, 16).T)
                d128_parts.append(seg_d.reshape(-1, P).T)
                off2 += c.nslots
        idx16 = np.tile(np.concatenate(i16_parts, axis=1), (8, 1))
        dl128 = np.concatenate(d128_parts, axis=1).astype(FP16)

        disb = np.zeros((P, WPD), np.float32)
        sl = slot_in_block[i * BLK : (i + 1) * BLK]
        disb[sl % P, sl // P] = dis[i * BLK : (i + 1) * BLK]

        in_maps.append(
            {
                "t1": T1,
                "tself": np.ascontiguousarray(
                    T1[i * BLKP : (i + 1) * BLKP]
                ).reshape(P, WPD * F),
                "idx16": idx16,
                "dl128": dl128,
                "disb": disb,
                "iota": iota_np,
                "ident": ident_np,
                "onesr": onesr_np,
                "w1": w1b,
                "w2": w2b,
                "w3": w3b,
                "b1f": b1f,
                "b2f": b2f,
                "b3f": b3f,
            }
        )

    unperm = np.empty(N, np.int64)
    unperm[:] = trow  # output row of node n within full [TROWS] layout
    meta["trow"] = trow
    return meta, in_maps


def _build_program(meta):
    import os
    import concourse.bacc as bacc
    import concourse.mybir as mybir
    import concourse.tile as tile
    from contextlib import ExitStack

    dbg_layers = int(os.environ.get("GNN_LAYERS", "3"))

    dt = mybir.dt
    nc = bacc.Bacc(
        "TRN2",
        target_bir_lowering=False,
        debug=False,
        num_devices=M,
        num_swdge_queues=4,
    )

    t1 = nc.dram_tensor("t1", [TROWS, F], dt.bfloat16, kind="ExternalInput")
    tselfd = nc.dram_tensor("tself", [P, WPD * F], dt.bfloat16, kind="ExternalInput")
    idxd = nc.dram_tensor("idx16", [P, meta["sc16"]], dt.int16, kind="ExternalInput")
    dld = nc.dram_tensor("dl128", [P, meta["sc128"]], dt.float16, kind="ExternalInput")
    disd = nc.dram_tensor("disb", [P, WPD], dt.float32, kind="ExternalInput")
    iod = nc.dram_tensor("iota", [P, WB, P], dt.float16, kind="ExternalInput")
    idnd = nc.dram_tensor("ident", [P, P], dt.bfloat16, kind="ExternalInput")
    onesd = nc.dram_tensor("onesr", [P, P], dt.bfloat16, kind="ExternalInput")
    w1d = nc.dram_tensor("w1", [F, F], dt.bfloat16, kind="ExternalInput")
    w2d = nc.dram_tensor("w2", [F, F], dt.bfloat16, kind="ExternalInput")
    w3d = nc.dram_tensor("w3", [F, DOUT], dt.bfloat16, kind="ExternalInput")
    b1d = nc.dram_tensor("b1f", [P, F], dt.bfloat16, kind="ExternalInput")
    b2d = nc.dram_tensor("b2f", [P, F], dt.bfloat16, kind="ExternalInput")
    b3d = nc.dram_tensor("b3f", [P, DOUT], dt.bfloat16, kind="ExternalInput")
    outd = nc.dram_tensor("out", [BLKP, DOUT], dt.float32, kind="ExternalOutput")

    wcnt_of = [WB] * 16 + [2]

    with tile.TileContext(nc) as tc, ExitStack() as ctx:
        const = ctx.enter_context(tc.tile_pool(name="const", bufs=1))
        dram = ctx.enter_context(tc.tile_pool(name="dram", bufs=1, space="DRAM"))
        ipool = ctx.enter_context(tc.tile_pool(name="ip", bufs=8))
        dpool = ctx.enter_context(tc.tile_pool(name="dp", bufs=8))
        gpool = ctx.enter_context(tc.tile_pool(name="gp", bufs=8))
        ohpool = ctx.enter_context(tc.tile_pool(name="ohp", bufs=8))
        upool = ctx.enter_context(tc.tile_pool(name="up", bufs=3))
        lhpool = ctx.enter_context(tc.tile_pool(name="lhp", bufs=3))
        zbpool = ctx.enter_context(tc.tile_pool(name="zbp", bufs=3))
        s0pool = ctx.enter_context(tc.tile_pool(name="s0p", bufs=2))
        stage = ctx.enter_context(tc.tile_pool(name="stage", bufs=2))
        apsum = ctx.enter_context(tc.tile_pool(name="apsum", bufs=4, space="PSUM"))
        tpsum = ctx.enter_context(tc.tile_pool(name="tpsum", bufs=2, space="PSUM"))
        zpsum = ctx.enter_context(tc.tile_pool(name="zpsum", bufs=2, space="PSUM"))

        def cload(name, dram_t, shape, dtype):
            tl = const.tile(shape, dtype, name=name)
            nc.sync.dma_start(out=tl[:], in_=dram_t[:])
            return tl

        iot = cload("iot", iod, [P, WB, P], dt.float16)
        idn = cload("idn", idnd, [P, P], dt.bfloat16)
        onesr = cload("onesr", onesd, [P, P], dt.bfloat16)
        dis_t = cload("dis_t", disd, [P, WPD], dt.float32)
        w1t = cload("w1t", w1d, [F, F], dt.bfloat16)
        w2t = cload("w2t", w2d, [F, F], dt.bfloat16)
        w3t = cload("w3t", w3d, [F, DOUT], dt.bfloat16)
        b1t = cload("b1t", b1d, [P, F], dt.bfloat16)
        b2t = cload("b2t", b2d, [P, F], dt.bfloat16)
        b3t = cload("b3t", b3d, [P, DOUT], dt.bfloat16)

        tin2 = dram.tile([BLKP, F], dt.bfloat16, name="tin2")
        tin3 = dram.tile([BLKP, F], dt.bfloat16, name="tin3")
        tf2 = dram.tile([TROWS, F], dt.bfloat16, addr_space="Shared", name="tf2")
        tf3 = dram.tile([TROWS, F], dt.bfloat16, addr_space="Shared", name="tf3")

        calls = meta["calls"]
        nmm = meta["nmm"]

        def do_layer(l, src_of, self_batch, wt, bt, tst, tin=None, tfull=None):
            mmctr = [0] * WPD
            for wb in range(NBATCH):
                w0 = wb * WB
                wcnt = wcnt_of[wb]
                self_of = self_batch(l, wb, w0, wcnt)
                gts, ohs, dts, its = [], [], [], []
                for p in range(NPAIR):
                    c = calls[(wb, p)]
                    it = ipool.tile([P, c.c16], dt.int16, tag="idx", name=f"it{l}_{wb}_{p}")
                    nc.sync.dma_start(out=it[:], in_=idxd[:, c.ic0 : c.ic0 + c.c16])
                    dt_ = dpool.tile(
                        [P, c.c128, 1], dt.float16, tag="dl", name=f"dl{l}_{wb}_{p}"
                    )
                    nc.sync.dma_start(
                        out=dt_[:],
                        in_=dld[:, c.dc0 : c.dc0 + c.c128].rearrange(
                            "p (c o) -> p c o", o=1
                        ),
                    )
                    gt = gpool.tile(
                        [P, c.c128, F], dt.bfloat16, tag="g", name=f"gt{l}_{wb}_{p}"
                    )
                    gts.append(gt)
                    dts.append(dt_)
                    its.append(it)
                # one gather per (batch, pair): saves ~0.8us fixed Q7 cost per
                # extra chunk (measured -670us Q7 busy). >64 descs/engine needs
                # per-descriptor packets (single_packet caps a packet at 64).
                for p in range(NPAIR):
                    c = calls[(wb, p)]
                    nc.gpsimd.dma_gather(
                        gts[p][:],
                        src_of(p),
                        its[p][:],
                        c.nslots,
                        c.nslots,
                        F,
                        queue_num=p,
                        single_packet=False,
                    )
                for p in range(NPAIR):
                    c = calls[(wb, p)]
                    oh = ohpool.tile(
                        [P, c.ohw, P], dt.bfloat16, tag="oh", name=f"oh{l}_{wb}_{p}"
                    )
                    for (w, glo, ghi, base) in c.slots:
                        run = ghi - glo
                        nc.vector.tensor_tensor(
                            out=oh[:, base : base + run, :],
                            in0=dts[p][:, glo:ghi, :].to_broadcast([P, run, P]),
                            in1=iot[:, w : w + 1, :].to_broadcast([P, run, P]),
                            op=mybir.AluOpType.is_equal,
                        )
                    ohs.append(oh)
                for wr in range(wcnt):
                    w = w0 + wr
                    agg = apsum.tile([P, F], dt.float32, tag="agg", name=f"agg{l}_{w}")
                    tot = int(nmm[w])
                    # self term: agg += I.T @ T_self[window w]
                    mmctr[w] += 1
                    nc.tensor.matmul(
                        agg[:],
                        lhsT=idn[:],
                        rhs=self_of(wr),
                        start=True,
                        stop=mmctr[w] == tot,
                    )
                    for p in range(NPAIR):
                        c = calls[(wb, p)]
                        for (ww, glo, ghi, base) in c.slots:
                            if ww != wr:
                                continue
                            for g in range(glo, ghi):
                                mmctr[w] += 1
                                nc.tensor.matmul(
                                    agg[:],
                                    lhsT=ohs[p][:, base + (g - glo), :],
                                    rhs=gts[p][:, g, :],
                                    start=False,
                                    stop=mmctr[w] == tot,
                                )
                    u = upool.tile([P, P], dt.bfloat16, tag="u", name=f"u{l}_{w}")
                    nc.scalar.activation(
                        u[:],
                        agg[:],
                        mybir.ActivationFunctionType.Copy,
                        scale=dis_t[:, w : w + 1],
                    )
                    tp = tpsum.tile([P, P], dt.bfloat16, tag="tp", name=f"tp{l}_{w}")
                    nc.tensor.transpose(tp[:], u[:], idn[:])
                    lh = lhpool.tile([P, P], dt.bfloat16, tag="lh", name=f"lh{l}_{w}")
                    nc.scalar.activation(
                        lh[:], tp[:], mybir.ActivationFunctionType.Copy
                    )
                    zw = zpsum.tile(
                        [P, F if l < 2 else DOUT], dt.float32, tag="zp", name=f"z{l}_{w}"
                    )
                    nc.tensor.matmul(zw[:], lhsT=lh[:], rhs=wt[:], start=True, stop=False)
                    # bias as rank-1 matmul: onesr row0 = ones, bt row0 = b
                    nc.tensor.matmul(zw[:], lhsT=onesr[:], rhs=bt[:], start=False, stop=True)
                    if l < 2:
                        nc.scalar.activation(
                            tst[:, w * F : (w + 1) * F],
                            zw[:],
                            mybir.ActivationFunctionType.Relu,
                            scale=dis_t[:, w : w + 1],
                        )
                    else:
                        nc.scalar.activation(
                            tst[:, w * DOUT : (w + 1) * DOUT],
                            zw[:],
                            mybir.ActivationFunctionType.Copy,
                        )
            if l < 2:
                # scalar-engine HWDGE queue: the sync queue is jammed with the
                # next layer's idx/dl prefetches (FIFO per engine), which would
                # delay this write and the collective behind it.
                nc.scalar.dma_start(
                    out=tin[:].rearrange("(p x) f -> p (x f)", p=P),
                    in_=tst[:],
                )
                nc.gpsimd.collective_compute(
                    "AllGather",
                    mybir.AluOpType.bypass,
                    replica_groups=[list(range(M))],
                    ins=[tin.opt()],
                    outs=[tfull.opt()],
                )
            else:
                nc.scalar.dma_start(
                    out=outd[:].rearrange("(p x) f -> p (x f)", p=P),
                    in_=tst[:],
                )

        # layer-1 self rows are loaded from DRAM per batch (not kept resident)
        def self_from_dram(l, wb, w0, wcnt):
            tb = s0pool.tile([P, wcnt * F], dt.bfloat16, tag="tb", name=f"tb{wb}")
            nc.sync.dma_start(out=tb[:], in_=tselfd[:, w0 * F : (w0 + wcnt) * F])
            return lambda wr: tb[:, wr * F : (wr + 1) * F]

        def self_from_stage(ts_prev):
            def f(l, wb, w0, wcnt):
                return lambda wr: ts_prev[:, (w0 + wr) * F : (w0 + wr + 1) * F]

            return f

        ts1 = stage.tile([P, WPD * F], dt.bfloat16, tag="tstage", name="ts1")
        do_layer(
            0,
            lambda p: t1[p * PAIR : (p + 1) * PAIR, :],
            self_from_dram,
            w1t,
            b1t,
            ts1,
            tin2,
            tf2,
        )
        if dbg_layers >= 2:
            ts2 = stage.tile([P, WPD * F], dt.bfloat16, tag="tstage", name="ts2")
            do_layer(
                1,
                lambda p: tf2[p * PAIR : (p + 1) * PAIR, :],
                self_from_stage(ts1),
                w2t,
                b2t,
                ts2,
                tin3,
                tf3,
            )
        if dbg_layers >= 3:
            ts3 = stage.tile([P, WPD * DOUT], dt.float32, tag="tstage", name="ts3")
            do_layer(
                2,
                lambda p: tf3[p * PAIR : (p + 1) * PAIR, :],
                self_from_stage(ts2),
                w3t,
                b3t,
                ts3,
            )
        else:
            zts = stage.tile([P, WPD * DOUT], dt.float32, tag="tstage", name="zts")
            nc.vector.memset(zts[:], 0.0)
            nc.sync.dma_start(
                out=outd[:].rearrange("(p x) f -> p (x f)", p=P),
                in_=zts[:],
            )

    nc.compile()
    return nc


_CACHE = {}


def _get_program(meta):
    import os

    key = (meta["sc16"], meta["sc128"], os.environ.get("GNN_LAYERS"))
    if key not in _CACHE:
        _CACHE[key] = _build_program(meta)
    return _CACHE[key]


def run(trace=False, **inputs):
    from concourse.bass_utils import run_bass_kernel_spmd

    meta, in_maps = _preprocess(**inputs)
    nc = _get_program(meta)
    res = run_bass_kernel_spmd(nc, in_maps, core_ids=list(range(M)), trace=trace)
    trow = meta["trow"]
    full = np.empty((TROWS, DOUT), np.float32)
    for i in range(M):
        full[i * BLKP : (i + 1) * BLKP] = res.results[i]["out"]
    out = full[trow]
    return out, res


def kernel(**inputs):
    out, _ = run(trace=False, **inputs)
    return out


# revision 35
# speedup vs baseline: 1.0193x; 1.0193x over previous
"""3-layer GCN (ContrastiveGNN) on 8 Trainium2 NeuronCores.

Strategy (dst-sharded edge partition, "1D graph partition"):
  - Nodes keep their original device (dev = node // 12500); within each device
    nodes are snake-dealt by in-degree into 17 batch bins (16 x 768 + 1 x 256
    slots) so per-(batch, src-pair) edge counts are balanced across devices.
  - Math reorder: for each GCN layer,
        out = D^-1/2 (A+I) D^-1/2 (h W) + b  ==  dis_d * (sum_{e->d} T[src]) @ W + b
    with T = dis * h (row-scaled activations).  Aggregation happens BEFORE the
    dense transform, so the gather tables carry 128 features for every layer.
  - Self-loops are NOT gathered: the self term dis_d*h[d] == T[d] is added into
    PSUM with one identity matmul per window, reading the device's own T rows
    (layer 1: a dedicated per-device input; layers 2-3: the resident stage tile
    holding the previous layer's dis-scaled output).
  - Aggregation on the tensor engine: per (batch, pair) all edges are packed
    densely sorted by dst window (NO per-window padding; groups of 128 edges
    may span window boundaries).  Each (group, window) intersection contributes
    one_hot[e, dst_in_batch==iota_w].T @ gathered[e, feat] accumulated in PSUM
    per 128-dst window.  One-hots are built on DVE via fp16 is_equal against
    per-window iota rows (values w_rel*128..w_rel*128+127).
  - Gathers use the SWDGE dma_gather custom instruction (int16 indices =>
    the 100352-row table is addressed in 4 block-pair regions of 25088 rows).
    One dma_gather per (batch, pair); pair p runs on SWDGE queue p so the four
    descriptor rings drain in parallel.
  - Tables are bf16 (PSUM accumulation f32); between layers the 8 per-device
    table blocks are exchanged with an AllGather collective.
  - All 8 devices run one SPMD program: per-call slot counts are padded to the
    max over devices, so instruction streams are identical and only the input
    data (indices, dst positions, dis) differs.
"""

import numpy as np
import ml_dtypes

BF16 = ml_dtypes.bfloat16
FP16 = np.float16

N = 100000
F = 128
DOUT = 64
M = 8
BLK = N // M            # 12500 dst nodes per device
P = 128
WPD = 98                # windows per device (98*128 = 12544)
BLKP = WPD * P          # 12544 padded block rows
TROWS = M * BLKP        # 100352 table rows
PAIR = 2 * BLKP         # 25088 rows per src-block-pair region (int16-addressable)
NPAIR = 4
WB = 6                  # windows per batch bin
NBATCH = 17             # 16 full bins (768 slots) + one 256-slot bin
BINFULL = WB * P        # 768
PADREL = PAIR - 1       # pair-local row of a guaranteed-zero table row


class _Call:
    __slots__ = ("ic0", "dc0", "c16", "c128", "nslots", "slots", "ohw")


def _preprocess(x, edge_index, W1, b1, W2, b2, W3, b3):
    """Host-side index plumbing + input staging."""
    x = np.asarray(x, np.float32)
    ei = np.asarray(edge_index)
    src = ei[0].astype(np.int64)
    dst = ei[1].astype(np.int64)

    deg = (np.bincount(dst, minlength=N) + 1).astype(np.float32)  # +1 self-loop
    dis = (1.0 / np.sqrt(deg)).astype(np.float32)

    # --- balanced slot assignment: dev fixed, snake-deal by degree into bins
    slot_in_block = np.empty(N, np.int64)
    nfull = 16 * BINFULL  # 12288
    for i in range(M):
        nodes = np.arange(i * BLK, (i + 1) * BLK)
        order = nodes[np.argsort(-deg[nodes], kind="stable")]
        main, tail = order[:nfull], order[nfull:]
        k = np.arange(nfull)
        fwd = (k // 16) % 2 == 0
        b = np.where(fwd, k % 16, 15 - (k % 16))
        slot_in_block[main] = b * BINFULL + (k // 16)
        slot_in_block[tail] = nfull + np.arange(len(tail))
    # lane-major table rows: row = lane*WPD + w, so the on-device stage tile
    # [P, WPD*F] maps to the table block with a straight contiguous copy.
    trow = (np.arange(N) // BLK) * BLKP + (slot_in_block % P) * WPD + (
        slot_in_block // P
    )

    # --- gather table for layer 1: dis-scaled input features
    xs = (x * dis[:, None]).astype(BF16)
    T1 = np.zeros((TROWS, F), BF16)
    T1[trow] = xs

    # --- edge bucketing (no self loops)
    e_dev = dst // BLK
    d_slot = slot_in_block[dst]
    e_batch = d_slot // BINFULL          # 0..16
    dl_all = d_slot - e_batch * BINFULL  # 0..767
    e_pair = (src // BLK) // 2
    rel_all = (trow[src] - e_pair * PAIR).astype(np.int64)
    bkey = e_batch * NPAIR + e_pair      # 0..67
    NBUCK = NBATCH * NPAIR

    cnt = np.zeros((M, NBUCK), np.int64)
    for i in range(M):
        cnt[i] = np.bincount(bkey[e_dev == i], minlength=NBUCK)
    cap = np.maximum(128, -(-cnt.max(axis=0) // P) * P)  # per-bucket slots

    # per-device window start/end within each bucket (slot positions)
    # windows per batch bin: WB except last bin (2)
    wcnt_of = [WB] * 16 + [2]

    meta_calls = {}
    ic = dc = off = 0
    win_starts = np.zeros((M, NBUCK, WB), np.int64)
    win_ends = np.zeros((M, NBUCK, WB), np.int64)

    # sort edges per device by (bucket, dl)
    dev_data = []
    for i in range(M):
        m = e_dev == i
        key = bkey[m] * BINFULL + dl_all[m]
        o = np.argsort(key, kind="stable")
        bk_s = bkey[m][o]
        dl_s = dl_all[m][o]
        rel_s = rel_all[m][o]
        starts = np.searchsorted(bk_s, np.arange(NBUCK))
        ends = np.searchsorted(bk_s, np.arange(NBUCK), side="right")
        for bu in range(NBUCK):
            s0, s1 = starts[bu], ends[bu]
            dseg = dl_s[s0:s1]
            wc = wcnt_of[bu // NPAIR]
            for w in range(wc):
                win_starts[i, bu, w] = np.searchsorted(dseg, w * P)
                win_ends[i, bu, w] = np.searchsorted(dseg, (w + 1) * P)
        dev_data.append((bk_s, dl_s, rel_s, starts, ends))

    # union group ranges + call metadata (SPMD-uniform)
    nmm = np.ones(WPD, np.int64)  # 1 self-matmul per window
    for wb in range(NBATCH):
        wc = wcnt_of[wb]
        for p_ in range(NPAIR):
            bu = wb * NPAIR + p_
            c = _Call()
            c.ic0, c.dc0 = ic, dc
            c.nslots = int(cap[bu])
            c.c16 = c.nslots // 16
            c.c128 = c.nslots // P
            slots = []
            base = 0
            for w in range(wc):
                glo = int(min(win_starts[i2, bu, w] for i2 in range(M)) // P)
                ghi_e = max(int(win_ends[i2, bu, w]) for i2 in range(M))
                ghi = min(-(-ghi_e // P), c.c128)
                ghi = max(ghi, glo)
                if ghi > glo:
                    slots.append((w, glo, ghi, base))
                    base += ghi - glo
                    nmm[wb * WB + w] += ghi - glo
            c.slots = slots
            c.ohw = base
            ic += c.c16
            dc += c.c128
            off += c.nslots
            meta_calls[(wb, p_)] = c
    tot_slots = off

    meta = {
        "calls": meta_calls,
        "nmm": nmm,
        "sc16": tot_slots // 16,
        "sc128": tot_slots // P,
        "tot_slots": tot_slots,
    }

    # --- per-device padded slot arrays
    iota_np = np.zeros((P, WB, P), FP16)
    for j in range(WB):
        iota_np[:, j, :] = np.arange(j * P, (j + 1) * P, dtype=np.float32)[None, :]
    ident_np = np.eye(P, dtype=np.float32).astype(BF16)
    onesr_np = np.zeros((P, P), BF16)
    onesr_np[0, :] = 1
    w1b = np.asarray(W1, np.float32).astype(BF16)
    w2b = np.asarray(W2, np.float32).astype(BF16)
    w3b = np.asarray(W3, np.float32).astype(BF16)
    # bias as a rank-1 matmul operand: row 0 holds b, other rows zero
    b1f = np.zeros((P, F), np.float32).astype(BF16)
    b1f[0] = np.asarray(b1, np.float32)
    b2f = np.zeros((P, F), np.float32).astype(BF16)
    b2f[0] = np.asarray(b2, np.float32)
    b3f = np.zeros((P, DOUT), np.float32).astype(BF16)
    b3f[0] = np.asarray(b3, np.float32)

    in_maps = []
    for i in range(M):
        bk_s, dl_s, rel_s, starts, ends = dev_data[i]
        idxfl = np.full(tot_slots, PADREL, np.int16)
        dlfl = np.full(tot_slots, -1.0, np.float32)
        off2 = 0
        for wb in range(NBATCH):
            for p_ in range(NPAIR):
                bu = wb * NPAIR + p_
                c = meta_calls[(wb, p_)]
                s0, s1 = starts[bu], ends[bu]
                n = s1 - s0
                idxfl[off2 : off2 + n] = rel_s[s0:s1].astype(np.int16)
                dlfl[off2 : off2 + n] = dl_s[s0:s1]
                off2 += c.nslots

        i16_parts, d128_parts = [], []
        off2 = 0
        for wb in range(NBATCH):
            for p_ in range(NPAIR):
                c = meta_calls[(wb, p_)]
                seg_i = idxfl[off2 : off2 + c.nslots]
                seg_d = dlfl[off2 : off2 + c.nslots]
                i16_parts.append(seg_i.reshape(-1, 16).T)
                d128_parts.append(seg_d.reshape(-1, P).T)
                off2 += c.nslots
        idx16 = np.tile(np.concatenate(i16_parts, axis=1), (8, 1))
        dl128 = np.concatenate(d128_parts, axis=1).astype(FP16)

        disb = np.zeros((P, WPD), np.float32)
        sl = slot_in_block[i * BLK : (i + 1) * BLK]
        disb[sl % P, sl // P] = dis[i * BLK : (i + 1) * BLK]

        in_maps.append(
            {
                "t1": T1,
                "tself": np.ascontiguousarray(
                    T1[i * BLKP : (i + 1) * BLKP]
                ).reshape(P, WPD * F),
                "idx16": idx16,
                "dl128": dl128,
                "disb": disb,
                "iota": iota_np,
                "ident": ident_np,
                "onesr": onesr_np,
                "w1": w1b,
                "w2": w2b,
                "w3": w3b,
                "b1f": b1f,
                "b2f": b2f,
                "b3f": b3f,
            }
        )

    unperm = np.empty(N, np.int64)
    unperm[:] = trow  # output row of node n within full [TROWS] layout
    meta["trow"] = trow
    return meta, in_maps


def _build_program(meta):
    import os
    import concourse.bacc as bacc
    import concourse.mybir as mybir
    import concourse.tile as tile
    from contextlib import ExitStack

    dbg_layers = int(os.environ.get("GNN_LAYERS", "3"))

    dt = mybir.dt
    nc = bacc.Bacc(
        "TRN2",
        target_bir_lowering=False,
        debug=False,
        num_devices=M,
        num_swdge_queues=4,
    )

    t1 = nc.dram_tensor("t1", [TROWS, F], dt.bfloat16, kind="ExternalInput")
    tselfd = nc.dram_tensor("tself", [P, WPD * F], dt.bfloat16, kind="ExternalInput")
    idxd = nc.dram_tensor("idx16", [P, meta["sc16"]], dt.int16, kind="ExternalInput")
    dld = nc.dram_tensor("dl128", [P, meta["sc128"]], dt.float16, kind="ExternalInput")
    disd = nc.dram_tensor("disb", [P, WPD], dt.float32, kind="ExternalInput")
    iod = nc.dram_tensor("iota", [P, WB, P], dt.float16, kind="ExternalInput")
    idnd = nc.dram_tensor("ident", [P, P], dt.bfloat16, kind="ExternalInput")
    onesd = nc.dram_tensor("onesr", [P, P], dt.bfloat16, kind="ExternalInput")
    w1d = nc.dram_tensor("w1", [F, F], dt.bfloat16, kind="ExternalInput")
    w2d = nc.dram_tensor("w2", [F, F], dt.bfloat16, kind="ExternalInput")
    w3d = nc.dram_tensor("w3", [F, DOUT], dt.bfloat16, kind="ExternalInput")
    b1d = nc.dram_tensor("b1f", [P, F], dt.bfloat16, kind="ExternalInput")
    b2d = nc.dram_tensor("b2f", [P, F], dt.bfloat16, kind="ExternalInput")
    b3d = nc.dram_tensor("b3f", [P, DOUT], dt.bfloat16, kind="ExternalInput")
    outd = nc.dram_tensor("out", [BLKP, DOUT], dt.float32, kind="ExternalOutput")

    wcnt_of = [WB] * 16 + [2]

    with tile.TileContext(nc) as tc, ExitStack() as ctx:
        const = ctx.enter_context(tc.tile_pool(name="const", bufs=1))
        dram = ctx.enter_context(tc.tile_pool(name="dram", bufs=1, space="DRAM"))
        ipool = ctx.enter_context(tc.tile_pool(name="ip", bufs=8))
        dpool = ctx.enter_context(tc.tile_pool(name="dp", bufs=8))
        gpool = ctx.enter_context(tc.tile_pool(name="gp", bufs=8))
        ohpool = ctx.enter_context(tc.tile_pool(name="ohp", bufs=8))
        upool = ctx.enter_context(tc.tile_pool(name="up", bufs=3))
        lhpool = ctx.enter_context(tc.tile_pool(name="lhp", bufs=3))
        zbpool = ctx.enter_context(tc.tile_pool(name="zbp", bufs=3))
        s0pool = ctx.enter_context(tc.tile_pool(name="s0p", bufs=2))
        stage = ctx.enter_context(tc.tile_pool(name="stage", bufs=2))
        apsum = ctx.enter_context(tc.tile_pool(name="apsum", bufs=4, space="PSUM"))
        tpsum = ctx.enter_context(tc.tile_pool(name="tpsum", bufs=2, space="PSUM"))
        zpsum = ctx.enter_context(tc.tile_pool(name="zpsum", bufs=2, space="PSUM"))

        def cload(name, dram_t, shape, dtype):
            tl = const.tile(shape, dtype, name=name)
            nc.sync.dma_start(out=tl[:], in_=dram_t[:])
            return tl

        iot = cload("iot", iod, [P, WB, P], dt.float16)
        idn = cload("idn", idnd, [P, P], dt.bfloat16)
        onesr = cload("onesr", onesd, [P, P], dt.bfloat16)
        dis_t = cload("dis_t", disd, [P, WPD], dt.float32)
        w1t = cload("w1t", w1d, [F, F], dt.bfloat16)
        w2t = cload("w2t", w2d, [F, F], dt.bfloat16)
        w3t = cload("w3t", w3d, [F, DOUT], dt.bfloat16)
        b1t = cload("b1t", b1d, [P, F], dt.bfloat16)
        b2t = cload("b2t", b2d, [P, F], dt.bfloat16)
        b3t = cload("b3t", b3d, [P, DOUT], dt.bfloat16)

        tin2 = dram.tile([BLKP, F], dt.bfloat16, name="tin2")
        tin3 = dram.tile([BLKP, F], dt.bfloat16, name="tin3")
        tf2 = dram.tile([TROWS, F], dt.bfloat16, addr_space="Shared", name="tf2")
        tf3 = dram.tile([TROWS, F], dt.bfloat16, addr_space="Shared", name="tf3")

        calls = meta["calls"]
        nmm = meta["nmm"]

        def do_layer(l, src_of, self_batch, wt, bt, tst, tin=None, tfull=None):
            mmctr = [0] * WPD
            for wb in range(NBATCH):
                w0 = wb * WB
                wcnt = wcnt_of[wb]
                self_of = self_batch(l, wb, w0, wcnt)
                gts, ohs, dts, its = [], [], [], []
                for p in range(NPAIR):
                    c = calls[(wb, p)]
                    it = ipool.tile([P, c.c16], dt.int16, tag="idx", name=f"it{l}_{wb}_{p}")
                    nc.sync.dma_start(out=it[:], in_=idxd[:, c.ic0 : c.ic0 + c.c16])
                    dt_ = dpool.tile(
                        [P, c.c128, 1], dt.float16, tag="dl", name=f"dl{l}_{wb}_{p}"
                    )
                    nc.sync.dma_start(
                        out=dt_[:],
                        in_=dld[:, c.dc0 : c.dc0 + c.c128].rearrange(
                            "p (c o) -> p c o", o=1
                        ),
                    )
                    gt = gpool.tile(
                        [P, c.c128, F], dt.bfloat16, tag="g", name=f"gt{l}_{wb}_{p}"
                    )
                    gts.append(gt)
                    dts.append(dt_)
                    its.append(it)
                # one gather per (batch, pair): saves ~0.8us fixed Q7 cost per
                # extra chunk (measured -670us Q7 busy). >64 descs/engine needs
                # per-descriptor packets (single_packet caps a packet at 64).
                for p in range(NPAIR):
                    c = calls[(wb, p)]
                    nc.gpsimd.dma_gather(
                        gts[p][:],
                        src_of(p),
                        its[p][:],
                        c.nslots,
                        c.nslots,
                        F,
                        queue_num=p,
                        single_packet=False,
                    )
                for p in range(NPAIR):
                    c = calls[(wb, p)]
                    oh = ohpool.tile(
                        [P, c.ohw, P], dt.bfloat16, tag="oh", name=f"oh{l}_{wb}_{p}"
                    )
                    for (w, glo, ghi, base) in c.slots:
                        run = ghi - glo
                        nc.vector.tensor_tensor(
                            out=oh[:, base : base + run, :],
                            in0=dts[p][:, glo:ghi, :].to_broadcast([P, run, P]),
                            in1=iot[:, w : w + 1, :].to_broadcast([P, run, P]),
                            op=mybir.AluOpType.is_equal,
                        )
                    ohs.append(oh)
                for wr in range(wcnt):
                    w = w0 + wr
                    agg = apsum.tile([P, F], dt.float32, tag="agg", name=f"agg{l}_{w}")
                    tot = int(nmm[w])
                    # self term: agg += I.T @ T_self[window w]
                    mmctr[w] += 1
                    nc.tensor.matmul(
                        agg[:],
                        lhsT=idn[:],
                        rhs=self_of(wr),
                        start=True,
                        stop=mmctr[w] == tot,
                    )
                    for p in range(NPAIR):
                        c = calls[(wb, p)]
                        for (ww, glo, ghi, base) in c.slots:
                            if ww != wr:
                                continue
                            for g in range(glo, ghi):
                                mmctr[w] += 1
                                nc.tensor.matmul(
                                    agg[:],
                                    lhsT=ohs[p][:, base + (g - glo), :],
                                    rhs=gts[p][:, g, :],
                                    start=False,
                                    stop=mmctr[w] == tot,
                                )
                    u = upool.tile([P, P], dt.bfloat16, tag="u", name=f"u{l}_{w}")
                    nc.vector.tensor_scalar(
                        u[:], agg[:], dis_t[:, w : w + 1], None, mybir.AluOpType.mult
                    )
                    tp = tpsum.tile([P, P], dt.bfloat16, tag="tp", name=f"tp{l}_{w}")
                    nc.tensor.transpose(tp[:], u[:], idn[:])
                    lh = lhpool.tile([P, P], dt.bfloat16, tag="lh", name=f"lh{l}_{w}")
                    nc.vector.tensor_copy(out=lh[:], in_=tp[:])
                    zw = zpsum.tile(
                        [P, F if l < 2 else DOUT], dt.float32, tag="zp", name=f"z{l}_{w}"
                    )
                    nc.tensor.matmul(zw[:], lhsT=lh[:], rhs=wt[:], start=True, stop=False)
                    # bias as rank-1 matmul: onesr row0 = ones, bt row0 = b
                    nc.tensor.matmul(zw[:], lhsT=onesr[:], rhs=bt[:], start=False, stop=True)
                    if l < 2:
                        nc.scalar.activation(
                            tst[:, w * F : (w + 1) * F],
                            zw[:],
                            mybir.ActivationFunctionType.Relu,
                            scale=dis_t[:, w : w + 1],
                        )
                    else:
                        nc.vector.tensor_copy(
                            out=tst[:, w * DOUT : (w + 1) * DOUT], in_=zw[:]
                        )
            if l < 2:
                # scalar-engine HWDGE queue: the sync queue is jammed with the
                # next layer's idx/dl prefetches (FIFO per engine), which would
                # delay this write and the collective behind it.
                nc.scalar.dma_start(
                    out=tin[:].rearrange("(p x) f -> p (x f)", p=P),
                    in_=tst[:],
                )
                nc.gpsimd.collective_compute(
                    "AllGather",
                    mybir.AluOpType.bypass,
                    replica_groups=[list(range(M))],
                    ins=[tin.opt()],
                    outs=[tfull.opt()],
                )
            else:
                nc.scalar.dma_start(
                    out=outd[:].rearrange("(p x) f -> p (x f)", p=P),
                    in_=tst[:],
                )

        # layer-1 self rows are loaded from DRAM per batch (not kept resident)
        def self_from_dram(l, wb, w0, wcnt):
            tb = s0pool.tile([P, wcnt * F], dt.bfloat16, tag="tb", name=f"tb{wb}")
            nc.sync.dma_start(out=tb[:], in_=tselfd[:, w0 * F : (w0 + wcnt) * F])
            return lambda wr: tb[:, wr * F : (wr + 1) * F]

        def self_from_stage(ts_prev):
            def f(l, wb, w0, wcnt):
                return lambda wr: ts_prev[:, (w0 + wr) * F : (w0 + wr + 1) * F]

            return f

        ts1 = stage.tile([P, WPD * F], dt.bfloat16, tag="tstage", name="ts1")
        do_layer(
            0,
            lambda p: t1[p * PAIR : (p + 1) * PAIR, :],
            self_from_dram,
            w1t,
            b1t,
            ts1,
            tin2,
            tf2,
        )
        if dbg_layers >= 2:
            ts2 = stage.tile([P, WPD * F], dt.bfloat16, tag="tstage", name="ts2")
            do_layer(
                1,
                lambda p: tf2[p * PAIR : (p + 1) * PAIR, :],
                self_from_stage(ts1),
                w2t,
                b2t,
                ts2,
                tin3,
                tf3,
            )
        if dbg_layers >= 3:
            ts3 = stage.tile([P, WPD * DOUT], dt.float32, tag="tstage", name="ts3")
            do_layer(
                2,
                lambda p: tf3[p * PAIR : (p + 1) * PAIR, :],
                self_from_stage(ts2),
                w3t,
                b3t,
                ts3,
            )
        else:
            zts = stage.tile([P, WPD * DOUT], dt.float32, tag="tstage", name="zts")
            nc.vector.memset(zts[:], 0.0)
            nc.sync.dma_start(
                out=outd[:].rearrange("(p x) f -> p (x f)", p=P),
                in_=zts[:],
            )

    nc.compile()
    return nc


_CACHE = {}


def _get_program(meta):
    import os

    key = (meta["sc16"], meta["sc128"], os.environ.get("GNN_LAYERS"))
    if key not in _CACHE:
        _CACHE[key] = _build_program(meta)
    return _CACHE[key]


def run(trace=False, **inputs):
    from concourse.bass_utils import run_bass_kernel_spmd

    meta, in_maps = _preprocess(**inputs)
    nc = _get_program(meta)
    res = run_bass_kernel_spmd(nc, in_maps, core_ids=list(range(M)), trace=trace)
    trow = meta["trow"]
    full = np.empty((TROWS, DOUT), np.float32)
    for i in range(M):
        full[i * BLKP : (i + 1) * BLKP] = res.results[i]["out"]
    out = full[trow]
    return out, res


def kernel(**inputs):
    out, _ = run(trace=False, **inputs)
    return out


# revision 41
# speedup vs baseline: 1.0636x; 1.0435x over previous
"""3-layer GCN (ContrastiveGNN) on 8 Trainium2 NeuronCores.

Strategy (dst-sharded edge partition, "1D graph partition"):
  - Nodes keep their original device (dev = node // 12500); within each device
    nodes are snake-dealt by in-degree into 17 batch bins (16 x 768 + 1 x 256
    slots) so per-(batch, src-pair) edge counts are balanced across devices.
  - Math reorder: for each GCN layer,
        out = D^-1/2 (A+I) D^-1/2 (h W) + b  ==  dis_d * (sum_{e->d} T[src]) @ W + b
    with T = dis * h (row-scaled activations).  Aggregation happens BEFORE the
    dense transform, so the gather tables carry 128 features for every layer.
  - Self-loops are NOT gathered: the self term dis_d*h[d] == T[d] is added into
    PSUM with one identity matmul per window, reading the device's own T rows
    (layer 1: a dedicated per-device input; layers 2-3: the resident stage tile
    holding the previous layer's dis-scaled output).
  - Aggregation on the tensor engine: per (batch, pair) all edges are packed
    densely sorted by dst window (NO per-window padding; groups of 128 edges
    may span window boundaries).  Each (group, window) intersection contributes
    one_hot[e, dst_in_batch==iota_w].T @ gathered[e, feat] accumulated in PSUM
    per 128-dst window.  One-hots are built on DVE via fp16 is_equal against
    per-window iota rows (values w_rel*128..w_rel*128+127).
  - Gathers use the SWDGE dma_gather custom instruction (int16 indices =>
    the 100352-row table is addressed in 4 block-pair regions of 25088 rows).
    One dma_gather per (batch, pair); pair p runs on SWDGE queue p so the four
    descriptor rings drain in parallel.
  - Tables are bf16 (PSUM accumulation f32); between layers the 8 per-device
    table blocks are exchanged with an AllGather collective.
  - All 8 devices run one SPMD program: per-call slot counts are padded to the
    max over devices, so instruction streams are identical and only the input
    data (indices, dst positions, dis) differs.
"""

import numpy as np
import ml_dtypes

BF16 = ml_dtypes.bfloat16
FP16 = np.float16

N = 100000
F = 128
DOUT = 64
M = 8
BLK = N // M            # 12500 dst nodes per device
P = 128
WPD = 98                # windows per device (98*128 = 12544)
BLKP = WPD * P          # 12544 padded block rows
TROWS = M * BLKP        # 100352 table rows
PAIR = 2 * BLKP         # 25088 rows per src-block-pair region (int16-addressable)
NPAIR = 4
WB = 6                  # windows per batch bin
NBATCH = 17             # 16 full bins (768 slots) + one 256-slot bin
BINFULL = WB * P        # 768
PADREL = PAIR - 1       # pair-local row of a guaranteed-zero table row


class _Call:
    __slots__ = ("ic0", "dc0", "c16", "c128", "nslots", "slots", "ohw")


def _preprocess(x, edge_index, W1, b1, W2, b2, W3, b3):
    """Host-side index plumbing + input staging."""
    x = np.asarray(x, np.float32)
    ei = np.asarray(edge_index)
    src = ei[0].astype(np.int64)
    dst = ei[1].astype(np.int64)

    deg = (np.bincount(dst, minlength=N) + 1).astype(np.float32)  # +1 self-loop
    dis = (1.0 / np.sqrt(deg)).astype(np.float32)

    # --- balanced slot assignment: dev fixed, snake-deal by degree into bins
    slot_in_block = np.empty(N, np.int64)
    nfull = 16 * BINFULL  # 12288
    for i in range(M):
        nodes = np.arange(i * BLK, (i + 1) * BLK)
        order = nodes[np.argsort(-deg[nodes], kind="stable")]
        main, tail = order[:nfull], order[nfull:]
        k = np.arange(nfull)
        fwd = (k // 16) % 2 == 0
        b = np.where(fwd, k % 16, 15 - (k % 16))
        slot_in_block[main] = b * BINFULL + (k // 16)
        slot_in_block[tail] = nfull + np.arange(len(tail))
    # lane-major table rows: row = lane*WPD + w, so the on-device stage tile
    # [P, WPD*F] maps to the table block with a straight contiguous copy.
    trow = (np.arange(N) // BLK) * BLKP + (slot_in_block % P) * WPD + (
        slot_in_block // P
    )

    # --- gather table for layer 1: dis-scaled input features
    xs = (x * dis[:, None]).astype(BF16)
    T1 = np.zeros((TROWS, F), BF16)
    T1[trow] = xs

    # --- edge bucketing (no self loops)
    e_dev = dst // BLK
    d_slot = slot_in_block[dst]
    e_batch = d_slot // BINFULL          # 0..16
    dl_all = d_slot - e_batch * BINFULL  # 0..767
    e_pair = (src // BLK) // 2
    rel_all = (trow[src] - e_pair * PAIR).astype(np.int64)
    bkey = e_batch * NPAIR + e_pair      # 0..67
    NBUCK = NBATCH * NPAIR

    cnt = np.zeros((M, NBUCK), np.int64)
    for i in range(M):
        cnt[i] = np.bincount(bkey[e_dev == i], minlength=NBUCK)
    cap = np.maximum(128, -(-cnt.max(axis=0) // P) * P)  # per-bucket slots

    # per-device window start/end within each bucket (slot positions)
    # windows per batch bin: WB except last bin (2)
    wcnt_of = [WB] * 16 + [2]

    meta_calls = {}
    ic = dc = off = 0
    win_starts = np.zeros((M, NBUCK, WB), np.int64)
    win_ends = np.zeros((M, NBUCK, WB), np.int64)

    # sort edges per device by (bucket, dl)
    dev_data = []
    for i in range(M):
        m = e_dev == i
        key = bkey[m] * BINFULL + dl_all[m]
        o = np.argsort(key, kind="stable")
        bk_s = bkey[m][o]
        dl_s = dl_all[m][o]
        rel_s = rel_all[m][o]
        starts = np.searchsorted(bk_s, np.arange(NBUCK))
        ends = np.searchsorted(bk_s, np.arange(NBUCK), side="right")
        for bu in range(NBUCK):
            s0, s1 = starts[bu], ends[bu]
            dseg = dl_s[s0:s1]
            wc = wcnt_of[bu // NPAIR]
            for w in range(wc):
                win_starts[i, bu, w] = np.searchsorted(dseg, w * P)
                win_ends[i, bu, w] = np.searchsorted(dseg, (w + 1) * P)
        dev_data.append((bk_s, dl_s, rel_s, starts, ends))

    # union group ranges + call metadata (SPMD-uniform)
    nmm = np.ones(WPD, np.int64)  # 1 self-matmul per window
    for wb in range(NBATCH):
        wc = wcnt_of[wb]
        for p_ in range(NPAIR):
            bu = wb * NPAIR + p_
            c = _Call()
            c.ic0, c.dc0 = ic, dc
            c.nslots = int(cap[bu])
            c.c16 = c.nslots // 16
            c.c128 = c.nslots // P
            slots = []
            base = 0
            for w in range(wc):
                glo = int(min(win_starts[i2, bu, w] for i2 in range(M)) // P)
                ghi_e = max(int(win_ends[i2, bu, w]) for i2 in range(M))
                ghi = min(-(-ghi_e // P), c.c128)
                ghi = max(ghi, glo)
                if ghi > glo:
                    slots.append((w, glo, ghi, base))
                    base += ghi - glo
                    nmm[wb * WB + w] += ghi - glo
            c.slots = slots
            c.ohw = base
            ic += c.c16
            dc += c.c128
            off += c.nslots
            meta_calls[(wb, p_)] = c
    tot_slots = off

    meta = {
        "calls": meta_calls,
        "nmm": nmm,
        "sc16": tot_slots // 16,
        "sc128": tot_slots // P,
        "tot_slots": tot_slots,
    }

    # --- per-device padded slot arrays
    iota_np = np.zeros((P, WB, P), FP16)
    for j in range(WB):
        iota_np[:, j, :] = np.arange(j * P, (j + 1) * P, dtype=np.float32)[None, :]
    ident_np = np.eye(P, dtype=np.float32).astype(BF16)
    w1b = np.asarray(W1, np.float32).astype(BF16)
    w2b = np.asarray(W2, np.float32).astype(BF16)
    w3b = np.asarray(W3, np.float32).astype(BF16)
    b1f = np.tile(np.asarray(b1, np.float32), (P, 1))
    b2f = np.tile(np.asarray(b2, np.float32), (P, 1))
    b3f = np.tile(np.asarray(b3, np.float32), (P, 1))

    in_maps = []
    for i in range(M):
        bk_s, dl_s, rel_s, starts, ends = dev_data[i]
        idxfl = np.full(tot_slots, PADREL, np.int16)
        dlfl = np.full(tot_slots, -1.0, np.float32)
        off2 = 0
        for wb in range(NBATCH):
            for p_ in range(NPAIR):
                bu = wb * NPAIR + p_
                c = meta_calls[(wb, p_)]
                s0, s1 = starts[bu], ends[bu]
                n = s1 - s0
                idxfl[off2 : off2 + n] = rel_s[s0:s1].astype(np.int16)
                dlfl[off2 : off2 + n] = dl_s[s0:s1]
                off2 += c.nslots

        i16_parts, d128_parts = [], []
        off2 = 0
        for wb in range(NBATCH):
            for p_ in range(NPAIR):
                c = meta_calls[(wb, p_)]
                seg_i = idxfl[off2 : off2 + c.nslots]
                seg_d = dlfl[off2 : off2 + c.nslots]
                i16_parts.append(seg_i.reshape(-1, 16).T)
                d128_parts.append(seg_d.reshape(-1, P).T)
                off2 += c.nslots
        idx16 = np.tile(np.concatenate(i16_parts, axis=1), (8, 1))
        dl128 = np.concatenate(d128_parts, axis=1).astype(FP16)

        disb = np.zeros((P, WPD), np.float32)
        sl = slot_in_block[i * BLK : (i + 1) * BLK]
        disb[sl % P, sl // P] = dis[i * BLK : (i + 1) * BLK]

        in_maps.append(
            {
                "t1": T1,
                "tself": np.ascontiguousarray(
                    T1[i * BLKP : (i + 1) * BLKP]
                ).reshape(P, WPD * F),
                "idx16": idx16,
                "dl128": dl128,
                "disb": disb,
                "iota": iota_np,
                "ident": ident_np,
                "w1": w1b,
                "w2": w2b,
                "w3": w3b,
                "b1f": b1f,
                "b2f": b2f,
                "b3f": b3f,
            }
        )

    unperm = np.empty(N, np.int64)
    unperm[:] = trow  # output row of node n within full [TROWS] layout
    meta["trow"] = trow
    return meta, in_maps


def _build_program(meta):
    import os
    import concourse.bacc as bacc
    import concourse.mybir as mybir
    import concourse.tile as tile
    from contextlib import ExitStack

    dbg_layers = int(os.environ.get("GNN_LAYERS", "3"))

    dt = mybir.dt
    nc = bacc.Bacc(
        "TRN2",
        target_bir_lowering=False,
        debug=False,
        num_devices=M,
        num_swdge_queues=4,
    )

    t1 = nc.dram_tensor("t1", [TROWS, F], dt.bfloat16, kind="ExternalInput")
    tselfd = nc.dram_tensor("tself", [P, WPD * F], dt.bfloat16, kind="ExternalInput")
    idxd = nc.dram_tensor("idx16", [P, meta["sc16"]], dt.int16, kind="ExternalInput")
    dld = nc.dram_tensor("dl128", [P, meta["sc128"]], dt.float16, kind="ExternalInput")
    disd = nc.dram_tensor("disb", [P, WPD], dt.float32, kind="ExternalInput")
    iod = nc.dram_tensor("iota", [P, WB, P], dt.float16, kind="ExternalInput")
    idnd = nc.dram_tensor("ident", [P, P], dt.bfloat16, kind="ExternalInput")
    w1d = nc.dram_tensor("w1", [F, F], dt.bfloat16, kind="ExternalInput")
    w2d = nc.dram_tensor("w2", [F, F], dt.bfloat16, kind="ExternalInput")
    w3d = nc.dram_tensor("w3", [F, DOUT], dt.bfloat16, kind="ExternalInput")
    b1d = nc.dram_tensor("b1f", [P, F], dt.float32, kind="ExternalInput")
    b2d = nc.dram_tensor("b2f", [P, F], dt.float32, kind="ExternalInput")
    b3d = nc.dram_tensor("b3f", [P, DOUT], dt.float32, kind="ExternalInput")
    outd = nc.dram_tensor("out", [BLKP, DOUT], dt.float32, kind="ExternalOutput")

    wcnt_of = [WB] * 16 + [2]

    with tile.TileContext(nc) as tc, ExitStack() as ctx:
        const = ctx.enter_context(tc.tile_pool(name="const", bufs=1))
        dram = ctx.enter_context(tc.tile_pool(name="dram", bufs=1, space="DRAM"))
        ipool = ctx.enter_context(tc.tile_pool(name="ip", bufs=8))
        dpool = ctx.enter_context(tc.tile_pool(name="dp", bufs=8))
        gpool = ctx.enter_context(tc.tile_pool(name="gp", bufs=8))
        ohpool = ctx.enter_context(tc.tile_pool(name="ohp", bufs=8))
        upool = ctx.enter_context(tc.tile_pool(name="up", bufs=3))
        lhpool = ctx.enter_context(tc.tile_pool(name="lhp", bufs=3))
        zbpool = ctx.enter_context(tc.tile_pool(name="zbp", bufs=3))
        s0pool = ctx.enter_context(tc.tile_pool(name="s0p", bufs=2))
        stage = ctx.enter_context(tc.tile_pool(name="stage", bufs=2))
        apsum = ctx.enter_context(tc.tile_pool(name="apsum", bufs=4, space="PSUM"))
        tpsum = ctx.enter_context(tc.tile_pool(name="tpsum", bufs=2, space="PSUM"))
        zpsum = ctx.enter_context(tc.tile_pool(name="zpsum", bufs=2, space="PSUM"))

        def cload(name, dram_t, shape, dtype):
            tl = const.tile(shape, dtype, name=name)
            nc.sync.dma_start(out=tl[:], in_=dram_t[:])
            return tl

        iot = cload("iot", iod, [P, WB, P], dt.float16)
        idn = cload("idn", idnd, [P, P], dt.bfloat16)
        dis_t = cload("dis_t", disd, [P, WPD], dt.float32)
        w1t = cload("w1t", w1d, [F, F], dt.bfloat16)
        w2t = cload("w2t", w2d, [F, F], dt.bfloat16)
        w3t = cload("w3t", w3d, [F, DOUT], dt.bfloat16)
        b1t = cload("b1t", b1d, [P, F], dt.float32)
        b2t = cload("b2t", b2d, [P, F], dt.float32)
        b3t = cload("b3t", b3d, [P, DOUT], dt.float32)

        tin2 = dram.tile([BLKP, F], dt.bfloat16, name="tin2")
        tin3 = dram.tile([BLKP, F], dt.bfloat16, name="tin3")
        tf2 = dram.tile([TROWS, F], dt.bfloat16, addr_space="Shared", name="tf2")
        tf3 = dram.tile([TROWS, F], dt.bfloat16, addr_space="Shared", name="tf3")

        calls = meta["calls"]
        nmm = meta["nmm"]

        def do_layer(l, src_of, self_batch, wt, bt, tst, tin=None, tfull=None):
            mmctr = [0] * WPD
            for wb in range(NBATCH):
                w0 = wb * WB
                wcnt = wcnt_of[wb]
                self_of = self_batch(l, wb, w0, wcnt)
                gts, ohs, dts, its = [], [], [], []
                for p in range(NPAIR):
                    c = calls[(wb, p)]
                    it = ipool.tile([P, c.c16], dt.int16, tag="idx", name=f"it{l}_{wb}_{p}")
                    nc.sync.dma_start(out=it[:], in_=idxd[:, c.ic0 : c.ic0 + c.c16])
                    dt_ = dpool.tile(
                        [P, c.c128, 1], dt.float16, tag="dl", name=f"dl{l}_{wb}_{p}"
                    )
                    nc.sync.dma_start(
                        out=dt_[:],
                        in_=dld[:, c.dc0 : c.dc0 + c.c128].rearrange(
                            "p (c o) -> p c o", o=1
                        ),
                    )
                    gt = gpool.tile(
                        [P, c.c128, F], dt.bfloat16, tag="g", name=f"gt{l}_{wb}_{p}"
                    )
                    gts.append(gt)
                    dts.append(dt_)
                    its.append(it)
                # one gather per (batch, pair): saves ~0.8us fixed Q7 cost per
                # extra chunk (measured -670us Q7 busy). >64 descs/engine needs
                # per-descriptor packets (single_packet caps a packet at 64).
                for p in range(NPAIR):
                    c = calls[(wb, p)]
                    nc.gpsimd.dma_gather(
                        gts[p][:],
                        src_of(p),
                        its[p][:],
                        c.nslots,
                        c.nslots,
                        F,
                        queue_num=p,
                        single_packet=False,
                    )
                for p in range(NPAIR):
                    c = calls[(wb, p)]
                    oh = ohpool.tile(
                        [P, c.ohw, P], dt.bfloat16, tag="oh", name=f"oh{l}_{wb}_{p}"
                    )
                    for (w, glo, ghi, base) in c.slots:
                        run = ghi - glo
                        nc.vector.tensor_tensor(
                            out=oh[:, base : base + run, :],
                            in0=dts[p][:, glo:ghi, :].to_broadcast([P, run, P]),
                            in1=iot[:, w : w + 1, :].to_broadcast([P, run, P]),
                            op=mybir.AluOpType.is_equal,
                        )
                    ohs.append(oh)
                for wr in range(wcnt):
                    w = w0 + wr
                    agg = apsum.tile([P, F], dt.float32, tag="agg", name=f"agg{l}_{w}")
                    tot = int(nmm[w])
                    # self term: agg += I.T @ T_self[window w]
                    mmctr[w] += 1
                    nc.tensor.matmul(
                        agg[:],
                        lhsT=idn[:],
                        rhs=self_of(wr),
                        start=True,
                        stop=mmctr[w] == tot,
                    )
                    for p in range(NPAIR):
                        c = calls[(wb, p)]
                        for (ww, glo, ghi, base) in c.slots:
                            if ww != wr:
                                continue
                            for g in range(glo, ghi):
                                mmctr[w] += 1
                                nc.tensor.matmul(
                                    agg[:],
                                    lhsT=ohs[p][:, base + (g - glo), :],
                                    rhs=gts[p][:, g, :],
                                    start=False,
                                    stop=mmctr[w] == tot,
                                )
                    u = upool.tile([P, P], dt.bfloat16, tag="u", name=f"u{l}_{w}")
                    nc.vector.tensor_scalar(
                        u[:], agg[:], dis_t[:, w : w + 1], None, mybir.AluOpType.mult
                    )
                    tp = tpsum.tile([P, P], dt.bfloat16, tag="tp", name=f"tp{l}_{w}")
                    nc.tensor.transpose(tp[:], u[:], idn[:])
                    lh = lhpool.tile([P, P], dt.bfloat16, tag="lh", name=f"lh{l}_{w}")
                    nc.vector.tensor_copy(out=lh[:], in_=tp[:])
                    zw = zpsum.tile(
                        [P, F if l < 2 else DOUT], dt.float32, tag="zp", name=f"z{l}_{w}"
                    )
                    nc.tensor.matmul(zw[:], lhsT=lh[:], rhs=wt[:], start=True, stop=True)
                    if l < 2:
                        zb = zbpool.tile([P, F], dt.float32, tag="zb", name=f"zb{l}_{w}")
                        nc.vector.tensor_tensor(
                            out=zb[:], in0=zw[:], in1=bt[:], op=mybir.AluOpType.add
                        )
                        nc.scalar.activation(
                            tst[:, w * F : (w + 1) * F],
                            zb[:],
                            mybir.ActivationFunctionType.Relu,
                            scale=dis_t[:, w : w + 1],
                        )
                    else:
                        nc.vector.tensor_tensor(
                            out=tst[:, w * DOUT : (w + 1) * DOUT],
                            in0=zw[:],
                            in1=bt[:],
                            op=mybir.AluOpType.add,
                        )
            if l < 2:
                # scalar-engine HWDGE queue: the sync queue is jammed with the
                # next layer's idx/dl prefetches (FIFO per engine), which would
                # delay this write and the collective behind it.
                nc.scalar.dma_start(
                    out=tin[:].rearrange("(p x) f -> p (x f)", p=P),
                    in_=tst[:],
                )
                nc.gpsimd.collective_compute(
                    "AllGather",
                    mybir.AluOpType.bypass,
                    replica_groups=[list(range(M))],
                    ins=[tin.opt()],
                    outs=[tfull.opt()],
                )
            else:
                nc.scalar.dma_start(
                    out=outd[:].rearrange("(p x) f -> p (x f)", p=P),
                    in_=tst[:],
                )

        # layer-1 self rows are loaded from DRAM per batch (not kept resident)
        def self_from_dram(l, wb, w0, wcnt):
            tb = s0pool.tile([P, wcnt * F], dt.bfloat16, tag="tb", name=f"tb{wb}")
            nc.sync.dma_start(out=tb[:], in_=tselfd[:, w0 * F : (w0 + wcnt) * F])
            return lambda wr: tb[:, wr * F : (wr + 1) * F]

        def self_from_stage(ts_prev):
            def f(l, wb, w0, wcnt):
                return lambda wr: ts_prev[:, (w0 + wr) * F : (w0 + wr + 1) * F]

            return f

        ts1 = stage.tile([P, WPD * F], dt.bfloat16, tag="tstage", name="ts1")
        do_layer(
            0,
            lambda p: t1[p * PAIR : (p + 1) * PAIR, :],
            self_from_dram,
            w1t,
            b1t,
            ts1,
            tin2,
            tf2,
        )
        if dbg_layers >= 2:
            ts2 = stage.tile([P, WPD * F], dt.bfloat16, tag="tstage", name="ts2")
            do_layer(
                1,
                lambda p: tf2[p * PAIR : (p + 1) * PAIR, :],
                self_from_stage(ts1),
                w2t,
                b2t,
                ts2,
                tin3,
                tf3,
            )
        if dbg_layers >= 3:
            ts3 = stage.tile([P, WPD * DOUT], dt.float32, tag="tstage", name="ts3")
            do_layer(
                2,
                lambda p: tf3[p * PAIR : (p + 1) * PAIR, :],
                self_from_stage(ts2),
                w3t,
                b3t,
                ts3,
            )
        else:
            zts = stage.tile([P, WPD * DOUT], dt.float32, tag="tstage", name="zts")
            nc.vector.memset(zts[:], 0.0)
            nc.sync.dma_start(
                out=outd[:].rearrange("(p x) f -> p (x f)", p=P),
                in_=zts[:],
            )

    nc.compile()
    return nc


_CACHE = {}


def _get_program(meta):
    import os

    key = (meta["sc16"], meta["sc128"], os.environ.get("GNN_LAYERS"))
    if key not in _CACHE:
        _CACHE[key] = _build_program(meta)
    return _CACHE[key]


def run(trace=False, **inputs):
    from concourse.bass_utils import run_bass_kernel_spmd

    meta, in_maps = _preprocess(**inputs)
    nc = _get_program(meta)
    res = run_bass_kernel_spmd(nc, in_maps, core_ids=list(range(M)), trace=trace)
    trow = meta["trow"]
    full = np.empty((TROWS, DOUT), np.float32)
    for i in range(M):
        full[i * BLKP : (i + 1) * BLKP] = res.results[i]["out"]
    out = full[trow]
    return out, res


def kernel(**inputs):
    out, _ = run(trace=False, **inputs)
    return out
